# revision 6
# baseline (speedup 1.0000x reference)
"""Trainium2 Bass kernel for nn_ConformHopfieldBatchSameEnc.

Per (b, m): q = LN(head_m(enc(X_true))), k = LN(head_m(enc(X_sim))),
Q = q@Wq, K = k@Wk (4 heads x 128), scoresT = K Q^T / sqrt(128) (k-major),
diag masked, softmax over k, out = attn^T v, losses[m] = mean (out - v)^2.

Sharding: batch across 8 cores -> 2 batches x 4 models = 8 pairs/core.
Layout: feature-major [feat<=128 partitions, 512 tokens].  Attention is
k-major; exp(scoresT) tiles are masked by precomputed {0,1} tiles (zeroing
the diagonal segment), then D = sum_k E and N = sum_k E*v are computed on
the PE with a (ones,v)-column stationary operand into one [8,512] psum
tile (D rows 0-3, N rows 4-7).  D/N ship to the host, which finishes
out = N/D and the tiny loss reduction.  LN affine (g,b) and the attention
scale are folded into Wq/Wk on the host.

Engine legality rules honored: engine APs use partition base 0 with step 1
(32-aligned bases allowed); every float32r matmul input is produced as a
declared-f32r tile (DMA from f32r DRAM, or ACT/DVE writing an f32r tile).
"""

import functools
import math
from contextlib import ExitStack

import numpy as np

import concourse.bacc as bacc
import concourse.tile as tile
from concourse import mybir
from concourse.bass_utils import run_bass_kernel_spmd

F32 = mybir.dt.float32
F32R = mybir.dt.float32r
AF = mybir.ActivationFunctionType
ALU = mybir.AluOpType

B, M, S, DIN, E_, DOUT, H, DH = 16, 4, 512, 64, 4, 128, 4, 128
HE, HH = 600, 200
LN_EPS = 1e-5
N_CORES = 8
B_PER_CORE = B // N_CORES
PAIRS = B_PER_CORE * M

ECH = [(120 * i, 120) for i in range(5)]
HCH = [(0, 128), (128, 72)]
SCALE = 1.0 / math.sqrt(float(DOUT))


def build_nc():
    nc = bacc.Bacc("TRN2", target_bir_lowering=False, debug=False,
                   enable_asserts=True, num_devices=N_CORES)

    def din(name, shape, dt=F32R):
        return nc.dram_tensor(name, shape, dt, kind="ExternalInput").ap()

    xq_d = din("xq", [PAIRS, 128, S])
    xk_d = din("xk", [PAIRS, 128, S])
    dnsel_d = din("dnsel", [PAIRS, 128, 128])   # 16 lhsT of [128,8] (kc,h)
    w1_d = din("w1", [128, HE])
    b1_d = din("b1c", [120, 5], F32)
    w2_d = din("w2", [HE, HE])
    b2_d = din("b2c", [120, 5], F32)
    w3_d = din("w3", [HE, DOUT])
    b3_d = din("b3c", [DOUT, 1], F32)
    hw1_d = din("hw1", [M, DOUT, HH])
    hb1_d = din("hb1c", [M, 128, 2], F32)
    hw2_d = din("hw2", [M, HH, HH])
    hb2_d = din("hb2c", [M, 128, 2], F32)
    hw3_d = din("hw3", [M, HH, DOUT])
    hb3_d = din("hb3c", [M, DOUT, 1], F32)
    wgq_d = din("wgq", [M, DOUT, H * DH])
    cbq_d = din("cbqc", [M, DH, H], F32)
    wgk_d = din("wgk", [M, DOUT, H * DH])
    cbk_d = din("cbkc", [M, DH, H], F32)
    stat_d = din("statc", [128, 2])             # col0=1/128, col1=1.0 (f32r)
    eps_d = din("epsc", [1, 1], F32)
    mask_d = din("maskc", [4, 128, S], F32)     # 0 on diag segment, else 1

    dn_d = nc.dram_tensor("dnout", [8 * PAIRS, S], F32, kind="ExternalOutput").ap()

    with tile.TileContext(nc) as tc, ExitStack() as ctx:
        wpool = ctx.enter_context(tc.tile_pool(name="weights", bufs=1))

        def load(dram_ap, shape, tag, dt=F32R):
            t = wpool.tile(shape, dt, tag=tag)
            nc.sync.dma_start(t[:], dram_ap)
            return t

        w1 = load(w1_d[:, :], [128, HE], "w1")
        b1 = load(b1_d[:, :], [120, 5], "b1", F32)
        w2 = [load(w2_d[o:o + n, :], [n, HE], f"w2_{i}")
              for i, (o, n) in enumerate(ECH)]
        b2 = load(b2_d[:, :], [120, 5], "b2", F32)
        w3 = [load(w3_d[o:o + n, :], [n, DOUT], f"w3_{i}")
              for i, (o, n) in enumerate(ECH)]
        b3 = load(b3_d[:, :], [DOUT, 1], "b3", F32)
        hw1 = [load(hw1_d[m], [DOUT, HH], f"hw1_{m}") for m in range(M)]
        hb1 = [load(hb1_d[m], [128, 2], f"hb1_{m}", F32) for m in range(M)]
        hw2 = [[load(hw2_d[m, o:o + n, :], [n, HH], f"hw2_{m}_{i}")
                for i, (o, n) in enumerate(HCH)] for m in range(M)]
        hb2 = [load(hb2_d[m], [128, 2], f"hb2_{m}", F32) for m in range(M)]
        hw3 = [[load(hw3_d[m, o:o + n, :], [n, DOUT], f"hw3_{m}_{i}")
                for i, (o, n) in enumerate(HCH)] for m in range(M)]
        hb3 = [load(hb3_d[m], [DOUT, 1], f"hb3_{m}", F32) for m in range(M)]
        wgq = [load(wgq_d[m], [DOUT, H * DH], f"wgq_{m}") for m in range(M)]
        cbq = [load(cbq_d[m], [DH, H], f"cbq_{m}", F32) for m in range(M)]
        wgk = [load(wgk_d[m], [DOUT, H * DH], f"wgk_{m}") for m in range(M)]
        cbk = [load(cbk_d[m], [DH, H], f"cbk_{m}", F32) for m in range(M)]
        statc = load(stat_d[:, :], [128, 2], "statc")
        epsc = load(eps_d[:, :], [1, 1], "epsc", F32)
        maskc = [load(mask_d[kc], [128, S], f"mask_{kc}", F32) for kc in range(4)]

        def mk(name, bufs):
            return ctx.enter_context(tc.tile_pool(name=name, bufs=bufs))

        px = mk("px", 4)
        pench = mk("pench", 10)
        pe3 = mk("pe3", 3)
        phead = mk("phead", 6)
        pg3 = mk("pg3", 3)
        psq = mk("psq", 2)
        pz1 = mk("pz1", 2)
        pz = mk("pz", 3)
        pqt = mk("pqt", 8)
        pe_ = mk("pet", 3)
        pem = mk("pem", 3)
        prow = mk("prow", 6)
        pbc = mk("pbc", 4)
        pdnin = mk("pdnin", 2)
        pdns = mk("pdns", 2)

        pmm = ctx.enter_context(tc.tile_pool(name="pmm", bufs=3, space="PSUM"))
        pscore = ctx.enter_context(tc.tile_pool(name="pscore", bufs=2, space="PSUM"))
        pdn = ctx.enter_context(tc.tile_pool(name="pdn", bufs=1, space="PSUM"))
        paux = ctx.enter_context(tc.tile_pool(name="paux", bufs=2, space="PSUM"))

        def ln_norm(g3):
            """g3 [128,S] f32r -> z [128,S] f32r, z = (g3 - mu)/sqrt(var+eps)."""
            sq = psq.tile([128, S], F32R, tag="sq")
            nc.vector.tensor_mul(sq[:, :], g3[:, :], g3[:, :])
            mu_ps = paux.tile([1, S], F32, tag="aux")
            nc.tensor.matmul(mu_ps[0:1, :], statc[:, 0:1], g3[:, :],
                             start=True, stop=True)
            msq_ps = paux.tile([1, S], F32, tag="aux")
            nc.tensor.matmul(msq_ps[0:1, :], statc[:, 0:1], sq[:, :],
                             start=True, stop=True)
            mu_s = prow.tile([1, S], F32, tag="row")
            nc.scalar.activation(mu_s[:, :], mu_ps[0:1, :], AF.Identity, scale=1.0)
            mu2 = prow.tile([1, S], F32, tag="row")
            nc.scalar.square(mu2[:, :], mu_ps[0:1, :])
            var = prow.tile([1, S], F32, tag="row")
            nc.vector.tensor_sub(var[:, :], msq_ps[0:1, :], mu2[:, :])
            sd = prow.tile([1, S], F32, tag="row")
            nc.scalar.activation(sd[:, :], var[:, :], AF.Sqrt,
                                 bias=epsc[0:1, 0:1], scale=1.0)
            rstd = prow.tile([1, S], F32, tag="row")
            nc.vector.reciprocal(rstd[:, :], sd[:, :])
            mrs = prow.tile([1, S], F32, tag="row")
            nc.vector.tensor_mul(mrs[:, :], mu_s[:, :], rstd[:, :])
            rst_b = pbc.tile([128, S], F32, tag="bc")
            nc.gpsimd.partition_broadcast(rst_b[:, :], rstd[0:1, :])
            mrs_b = pbc.tile([128, S], F32, tag="bc")
            nc.gpsimd.partition_broadcast(mrs_b[:, :], mrs[0:1, :])
            z1 = pz1.tile([128, S], F32, tag="z1")
            nc.vector.tensor_mul(z1[:, :], g3[:, :], rst_b[:, :])
            z = pz.tile([128, S], F32R, tag="z")
            nc.vector.tensor_sub(z[:, :], z1[:, :], mrs_b[:, :])
            return z

        for p in range(PAIRS):
            m = p % M
            xq = px.tile([128, S], F32R, tag="x")
            nc.sync.dma_start(xq[:, :], xq_d[p])
            xk = px.tile([128, S], F32R, tag="x")
            nc.sync.dma_start(xk[:, :], xk_d[p])
            dnsel = pdnin.tile([128, 128], F32R, tag="dnsel")
            nc.sync.dma_start(dnsel[:, :], dnsel_d[p])

            zz = []
            for x, wg, cb in ((xq, wgq, cbq), (xk, wgk, cbk)):
                # encoder L1 (ACT relu+bias)
                h1 = []
                for j, (o, n) in enumerate(ECH):
                    ps = pmm.tile([128, S], F32, tag="mm")
                    nc.tensor.matmul(ps[:n, :], w1[0:DIN, o:o + n], x[0:DIN, :],
                                     start=True, stop=True)
                    t = pench.tile([120, S], F32R, tag="ench")
                    nc.scalar.activation(t[:n, :], ps[:n, :], AF.Relu,
                                         bias=b1[:n, j:j + 1], scale=1.0)
                    h1.append(t)
                # encoder L2 (DVE relu+bias: (x add b) max 0)
                h2 = []
                for j, (o, n) in enumerate(ECH):
                    ps = pmm.tile([128, S], F32, tag="mm")
                    for kc, (ko, kn) in enumerate(ECH):
                        nc.tensor.matmul(ps[:n, :], w2[kc][:kn, o:o + n],
                                         h1[kc][:kn, :],
                                         start=(kc == 0), stop=(kc == 4))
                    t = pench.tile([120, S], F32R, tag="ench")
                    nc.vector.tensor_scalar(t[:n, :], ps[:n, :],
                                            scalar1=b2[:n, j:j + 1], scalar2=0.0,
                                            op0=ALU.add, op1=ALU.max)
                    h2.append(t)
                # encoder L3
                ps = pmm.tile([128, S], F32, tag="mm")
                for kc, (ko, kn) in enumerate(ECH):
                    nc.tensor.matmul(ps[:, :], w3[kc][:kn, :], h2[kc][:kn, :],
                                     start=(kc == 0), stop=(kc == 4))
                e3 = pe3.tile([128, S], F32R, tag="e3")
                nc.scalar.activation(e3[:, :], ps[:, :], AF.Identity,
                                     bias=b3[:, 0:1], scale=1.0)
                # head L1 (ACT)
                g1 = []
                for j, (o, n) in enumerate(HCH):
                    ps = pmm.tile([128, S], F32, tag="mm")
                    nc.tensor.matmul(ps[:n, :], hw1[m][:, o:o + n], e3[:, :],
                                     start=True, stop=True)
                    t = phead.tile([128, S], F32R, tag="head")
                    nc.scalar.activation(t[:n, :], ps[:n, :], AF.Relu,
                                         bias=hb1[m][:n, j:j + 1], scale=1.0)
                    g1.append(t)
                # head L2 (DVE)
                g2 = []
                for j, (o, n) in enumerate(HCH):
                    ps = pmm.tile([128, S], F32, tag="mm")
                    for kc, (ko, kn) in enumerate(HCH):
                        nc.tensor.matmul(ps[:n, :], hw2[m][kc][:kn, o:o + n],
                                         g1[kc][:kn, :],
                                         start=(kc == 0), stop=(kc == 1))
                    t = phead.tile([128, S], F32R, tag="head")
                    nc.vector.tensor_scalar(t[:n, :], ps[:n, :],
                                            scalar1=hb2[m][:n, j:j + 1], scalar2=0.0,
                                            op0=ALU.add, op1=ALU.max)
                    g2.append(t)
                # head L3
                ps = pmm.tile([128, S], F32, tag="mm")
                for kc, (ko, kn) in enumerate(HCH):
                    nc.tensor.matmul(ps[:, :], hw3[m][kc][:kn, :], g2[kc][:kn, :],
                                     start=(kc == 0), stop=(kc == 1))
                g3 = pg3.tile([128, S], F32R, tag="g3")
                nc.scalar.activation(g3[:, :], ps[:, :], AF.Identity,
                                     bias=hb3[m][:, 0:1], scale=1.0)
                z = ln_norm(g3)
                # Q/K projection: per head [DH, S], DVE psum->sbuf copy
                qs = []
                for h in range(H):
                    ps = pmm.tile([128, S], F32, tag="mm")
                    nc.tensor.matmul(ps[:, :], wg[m][:, DH * h:DH * (h + 1)],
                                     z[:, :], start=True, stop=True)
                    t = pqt.tile([DH, S], F32R, tag="qt")
                    nc.scalar.activation(t[:, :], ps[:, :], AF.Identity,
                                         bias=cb[m][:, h:h + 1], scale=1.0)
                    qs.append(t)
                zz.append(qs)
            qt, kt = zz

            # ---- attention (k-major) + D/N contraction -------------------
            pdn_t = pdn.tile([8, S], F32, tag="dn")
            for kc in range(4):
                for h in range(H):
                    ps = pscore.tile([128, S], F32, tag="score")
                    nc.tensor.matmul(ps[:, :], kt[h][:, 128 * kc:128 * (kc + 1)],
                                     qt[h][:, :], start=True, stop=True)
                    et = pe_.tile([128, S], F32, tag="et")
                    nc.scalar.activation(et[:, :], ps[:, :], AF.Exp)
                    em = pem.tile([128, S], F32R, tag="em")
                    nc.vector.tensor_mul(em[:, :], et[:, :], maskc[kc][:, :])
                    nc.tensor.matmul(pdn_t[0:8, :],
                                     dnsel[:, 8 * (4 * kc + h):8 * (4 * kc + h) + 8],
                                     em[:, :],
                                     start=(kc == 0 and h == 0),
                                     stop=(kc == 3 and h == 3))
            dn_s = pdns.tile([8, S], F32, tag="dns")
            nc.vector.tensor_copy(dn_s[:, :], pdn_t[0:8, :])
            nc.sync.dma_start(dn_d[8 * p:8 * p + 8, :], dn_s[:, :])

    nc.compile()
    return nc


@functools.lru_cache(maxsize=1)
def get_nc():
    return build_nc()


def prep_inputs(inputs):
    f = {k: np.asarray(v, dtype=np.float32) if np.asarray(v).dtype.kind == "f"
         else np.asarray(v) for k, v in inputs.items()}
    wo = int(np.asarray(inputs["which_out"]))
    v = f["errors"][..., wo]  # [B, M, S]
    sq = np.float32(math.sqrt(SCALE))

    shared = {}
    w1 = f["enc_W1"]
    shared["w1"] = np.concatenate([w1, w1], axis=0).astype(np.float32)
    shared["b1c"] = np.stack([f["enc_b1"][o:o + n] for o, n in ECH], axis=1)
    shared["w2"] = f["enc_W2"]
    shared["b2c"] = np.stack([f["enc_b2"][o:o + n] for o, n in ECH], axis=1)
    shared["w3"] = f["enc_W3"]
    shared["b3c"] = f["enc_b3"][:, None]
    shared["hw1"] = f["hW1"]
    hb1c = np.zeros((M, 128, 2), np.float32)
    hb1c[:, 0:128, 0] = f["hb1"][:, 0:128]
    hb1c[:, 0:72, 1] = f["hb1"][:, 128:200]
    shared["hb1c"] = hb1c
    shared["hw2"] = f["hW2"]
    hb2c = np.zeros((M, 128, 2), np.float32)
    hb2c[:, 0:128, 0] = f["hb2"][:, 0:128]
    hb2c[:, 0:72, 1] = f["hb2"][:, 128:200]
    shared["hb2c"] = hb2c
    shared["hw3"] = f["hW3"]
    shared["hb3c"] = f["hb3"][:, :, None]
    shared["wgq"] = (f["Wq"] * f["lnq_g"][:, :, None] * sq).astype(np.float32)
    cbq = np.einsum("mo,moe->me", f["lnq_b"], f["Wq"]) * sq
    shared["cbqc"] = cbq.reshape(M, H, DH).transpose(0, 2, 1).astype(np.float32)
    shared["wgk"] = (f["Wk"] * f["lnk_g"][:, :, None] * sq).astype(np.float32)
    cbk = np.einsum("mo,moe->me", f["lnk_b"], f["Wk"]) * sq
    shared["cbkc"] = cbk.reshape(M, H, DH).transpose(0, 2, 1).astype(np.float32)
    statc = np.zeros((128, 2), np.float32)
    statc[:, 0] = 1.0 / 128.0
    statc[:, 1] = 1.0
    shared["statc"] = statc
    shared["epsc"] = np.full((1, 1), LN_EPS, np.float32)
    maskc = np.ones((4, 128, S), np.float32)
    for kc in range(4):
        for pp in range(128):
            maskc[kc, pp, 128 * kc + pp] = 0.0
    shared["maskc"] = maskc

    per_core = []
    for c in range(N_CORES):
        mp = {}
        xq = np.zeros((PAIRS, 128, S), np.float32)
        xk = np.zeros((PAIRS, 128, S), np.float32)
        dnsel = np.zeros((PAIRS, 128, 128), np.float32)
        for p in range(PAIRS):
            bl, m = divmod(p, M)
            b = B_PER_CORE * c + bl
            xt = f["X_true"][b, m].T
            xq[p] = np.concatenate([xt, xt], axis=0)
            xs = f["X_sim"][b, m].T
            xk[p] = np.concatenate([xs, xs], axis=0)
            vv = v[b, m]
            for kc in range(4):
                for h in range(H):
                    col = 8 * (4 * kc + h)
                    dnsel[p, :, col + h] = 1.0
                    dnsel[p, :, col + 4 + h] = vv[128 * kc:128 * (kc + 1)]
        mp["xq"], mp["xk"], mp["dnsel"] = xq, xk, dnsel
        mp.update(shared)
        per_core.append(mp)
    return per_core


def reduce_output(dns, inputs):
    """dns: 8 arrays [8*PAIRS, S] of stacked (D rows 0-3, N rows 4-7)."""
    f_err = np.asarray(inputs["errors"], dtype=np.float64)
    wo = int(np.asarray(inputs["which_out"]))
    v = f_err[..., wo]  # [B, M, S]
    losses = np.zeros(M, np.float64)
    for c in range(N_CORES):
        dn = np.asarray(dns[c], dtype=np.float64)
        for p in range(PAIRS):
            bl, m = divmod(p, M)
            b = B_PER_CORE * c + bl
            D = dn[8 * p:8 * p + 4]      # [H, S]
            N = dn[8 * p + 4:8 * p + 8]  # [H, S]
            out = N / D
            losses[m] += ((out - v[b, m][None, :]) ** 2).sum()
    return (losses / (B * S * H)).astype(np.float32)


def kernel(**inputs):
    nc = get_nc()
    per_core = prep_inputs(inputs)
    res = run_bass_kernel_spmd(nc, per_core, core_ids=list(range(N_CORES)))
    return reduce_output([res.results[c]["dnout"] for c in range(N_CORES)], inputs)
